# revision 3
# baseline (speedup 1.0000x reference)
"""GQA causal attention (RoPE) for TRN2, 8-core data+tensor parallel.

Sharding: core c in [0,8) handles batch b = c//4 and kv-head group g = c%4
(kv heads {2g, 2g+1}, q heads {4g..4g+3}).  wq/wk/wv column-sharded,
wo row-sharded by head group; host sums the 4 partial wo outputs per batch.

Device layouts (feature-major, "T" = transposed vs reference):
  xT   [DIM, S]      activations, d on partitions
  QT/KT [128, S]     per head (head_dim on partitions)
  V    [128k, 256]   natural (position on partitions), 16 k-tiles
  scoresT[k, q]      softmax denominator = partition-dim sum (ones matmul)
  attnT [128d, S]    per head -> wo matmul -> outT [DIM, S] bf16

Schedule: K/V projections for all 4 s-chunks first, then Q chunk 3, then
attention chunks in order [3,2,1,0].  The remaining Q-chunk projections and
the wo output tiles are emitted as *fillers* woven into the attention kt
loop, so the PE never idles on the exp/normalize latency chains.  Scores
psum tiles pair two k-tiles (two banks) so one ACT exp covers both.  The
softmax reciprocal is broadcast across partitions with a rank-1 matmul
(ones-column x recip-row) instead of a DRAM round trip.
"""

import json

import numpy as np
import ml_dtypes

import concourse.bass as bass
import concourse.mybir as mybir
import concourse.tile as tile
import concourse.bass2jax as bass2jax
import concourse.bass_utils as bass_utils
from concourse.bass_utils import run_bass_kernel_spmd


def _split_waits(bir_json: bytes) -> bytes:
    """This walrus build accepts at most ONE sync-wait per instruction (any
    opcode). Tile emits up to ~11. Hoist excess waits onto single-wait Drain
    fillers inserted just before the instruction on the same engine —
    same-engine program order makes this semantically identical."""
    j = json.loads(bir_json)
    changed = False
    for fn in j["functions"]:
        for b in fn["blocks"]:
            out = []
            for ins in b["instructions"]:
                si = ins.get("sync_info")
                ow = si.get("on_wait") if si else None
                if ow and len(ow) > 1:
                    changed = True
                    for k, w in enumerate(ow[:-1]):
                        out.append({
                            "debug": ins.get("debug", 0),
                            "engine": ins["engine"],
                            "ins": [], "outs": [],
                            "name": f"{ins['name']}-w{k}",
                            "opcode": "Drain",
                            "is_reset_sema": False,
                            "sync_info": {"on_update": [], "on_wait": [w]},
                        })
                    si["on_wait"] = [ow[-1]]
                out.append(ins)
            b["instructions"] = out
    return json.dumps(j).encode() if changed else bir_json


_ORIG_COMPILE = bass_utils.compile_bir_kernel


def _patched_compile(bir_json, tmpdir, neff_name="file.neff"):
    return _ORIG_COMPILE(_split_waits(bir_json), tmpdir, neff_name=neff_name)


if getattr(bass2jax.compile_bir_kernel, "__name__", "") != "_patched_compile":
    bass2jax.compile_bir_kernel = _patched_compile
    bass_utils.compile_bir_kernel = _patched_compile

BF16 = mybir.dt.bfloat16
F32 = mybir.dt.float32
Exp = mybir.ActivationFunctionType.Exp

B, S, DIM = 2, 2048, 2048
N_HEADS, N_KV_HEADS = 16, 8
HEAD_DIM, HALF = 128, 64
N_CORES = 8
QH, KVH = 4, 2            # q / kv heads per core
QW, KW = QH * HEAD_DIM, KVH * HEAD_DIM   # 512, 256
SCALE = 1.0 / float(np.sqrt(HEAD_DIM))

DT = DIM // 128           # 16 contraction tiles for projections
NSC = S // 512            # 4 s-chunks
NKT = S // 128            # 16 k tiles
NET = DIM // 128          # 16 output-feature tiles

_BUILT = {}


class Fillers:
    """FIFO of generators; each next() emits ~one PE matmul (plus attendant
    ACT/DVE/DMA ops).  feed(n) advances n steps; woven into the attention
    loop so the PE stream always has independent work behind a dependency."""

    def __init__(self):
        self.gens = []

    def add(self, gen):
        self.gens.append(gen)
        return gen

    def feed(self, n):
        while n > 0 and self.gens:
            try:
                next(self.gens[0])
                n -= 1
            except StopIteration:
                self.gens.pop(0)

    def drain(self, gen):
        if gen in self.gens:
            for _ in gen:
                pass
            self.gens.remove(gen)

    def drain_all(self):
        while self.gens:
            self.drain(self.gens[0])


def _build(nc):
    xt = nc.dram_tensor("xt", [DIM, S], BF16, kind="ExternalInput").ap()
    wq = nc.dram_tensor("wq", [DIM, QW], BF16, kind="ExternalInput").ap()
    wk = nc.dram_tensor("wk", [DIM, KW], BF16, kind="ExternalInput").ap()
    wv = nc.dram_tensor("wv", [DIM, KW], BF16, kind="ExternalInput").ap()
    wo = nc.dram_tensor("wo", [QW, DIM], BF16, kind="ExternalInput").ap()
    cosb = nc.dram_tensor("cosb", [HEAD_DIM, S], BF16, kind="ExternalInput").ap()
    sinb = nc.dram_tensor("sinb", [HEAD_DIM, S], F32, kind="ExternalInput").ap()
    pswp = nc.dram_tensor("pswp", [HEAD_DIM, HEAD_DIM], BF16, kind="ExternalInput").ap()
    tri = nc.dram_tensor("tri", [HEAD_DIM, HEAD_DIM], BF16, kind="ExternalInput").ap()
    ones = nc.dram_tensor("ones", [HEAD_DIM, 1], BF16, kind="ExternalInput").ap()
    onesr = nc.dram_tensor("onesr", [1, HEAD_DIM], BF16, kind="ExternalInput").ap()
    outT = nc.dram_tensor("outT", [DIM, S], BF16, kind="ExternalOutput").ap()

    with tile.TileContext(nc) as tc:
        with (
            tc.tile_pool(name="persist", bufs=1) as pp,
            tc.tile_pool(name="trans", bufs=2) as tp,
        ):
            # ---- DMA emission order = queue order = first-use order:
            # wk + x chunk0 (K proj starts the kernel), tables, wv,
            # x chunks 1-3, wq (first used at Q3), wo (used as fillers).
            xts = [[None] * DT for _ in range(NSC)]
            wk_sb, wv_sb, wq_sb, wo_sb = [], [], [], []
            for d in range(DT):
                t = pp.tile([128, KW], BF16, tag=f"wk{d}", name=f"wk_sb{d}")
                nc.sync.dma_start(t[:], wk[d * 128:(d + 1) * 128, :])
                wk_sb.append(t)
                xt_t = pp.tile([128, 512], BF16, tag=f"x0_{d}", name=f"xts0_{d}")
                nc.sync.dma_start(xt_t[:], xt[d * 128:(d + 1) * 128, 0:512])
                xts[0][d] = xt_t
            pswp_sb = pp.tile([HEAD_DIM, HEAD_DIM], BF16, tag="pswp", name="pswp_sb")
            nc.sync.dma_start(pswp_sb[:], pswp[:])
            tri_sb = pp.tile([HEAD_DIM, HEAD_DIM], BF16, tag="tri", name="tri_sb")
            nc.sync.dma_start(tri_sb[:], tri[:])
            ones_sb = pp.tile([HEAD_DIM, 1], BF16, tag="ones", name="ones_sb")
            nc.sync.dma_start(ones_sb[:], ones[:])
            onesr_sb = pp.tile([1, HEAD_DIM], BF16, tag="onesr", name="onesr_sb")
            nc.sync.dma_start(onesr_sb[:], onesr[:])
            cos_sb = pp.tile([HEAD_DIM, S], BF16, tag="cos", name="cos_sb")
            nc.sync.dma_start(cos_sb[:], cosb[:])
            sin_sb = pp.tile([HEAD_DIM, S], F32, tag="sin", name="sin_sb")
            nc.sync.dma_start(sin_sb[:], sinb[:])
            for d in range(DT):
                t = pp.tile([128, KW], BF16, tag=f"wv{d}", name=f"wv_sb{d}")
                nc.sync.dma_start(t[:], wv[d * 128:(d + 1) * 128, :])
                wv_sb.append(t)
            for sc in range(1, NSC):
                for d in range(DT):
                    xt_t = pp.tile([128, 512], BF16, tag=f"x{sc}_{d}",
                                   name=f"xts{sc}_{d}")
                    nc.sync.dma_start(
                        xt_t[:], xt[d * 128:(d + 1) * 128, sc * 512:(sc + 1) * 512])
                    xts[sc][d] = xt_t
            for d in range(DT):
                t = pp.tile([128, QW], BF16, tag=f"wq{d}", name=f"wq_sb{d}")
                nc.sync.dma_start(t[:], wq[d * 128:(d + 1) * 128, :])
                wq_sb.append(t)
            for h in range(QH):
                t = pp.tile([128, DIM], BF16, tag=f"wo{h}", name=f"wo_sb{h}")
                nc.sync.dma_start(t[:], wo[h * 128:(h + 1) * 128, :])
                wo_sb.append(t)

            # persistent intermediates
            qtr = [pp.tile([128, S], BF16, tag=f"qtr{h}", name=f"qtr{h}") for h in range(QH)]
            ktr = [pp.tile([128, S], BF16, tag=f"ktr{k}", name=f"ktr{k}") for k in range(KVH)]
            v_sb = [pp.tile([128, KW], BF16, tag=f"v{st}", name=f"v{st}") for st in range(NKT)]
            attnT = [pp.tile([128, S], BF16, tag=f"attnT{h}", name=f"attnT{h}") for h in range(QH)]

            # ============ Phase A: K/V all chunks, then Q chunk 3 ============
            with (
                tc.tile_pool(name="pA", bufs=3, space="PSUM") as pA,
                tc.tile_pool(name="pAv", bufs=2, space="PSUM") as pAv,
                tc.tile_pool(name="prm", bufs=2, space="PSUM") as prm,
            ):
                def rope_chunk(src, dst, ssl, pool):
                    """dst[:, ssl] = src*cos + pairswap(src)*sin~ ; src SBUF."""
                    shp = pool.tile([128, 512], F32, tag="rm", name="shp")
                    nc.tensor.matmul(shp[:], pswp_sb[:], src[:], start=True, stop=True)
                    t1 = tp.tile([128, 512], BF16, tag="t1", bufs=3, name="rope_t1")
                    nc.vector.tensor_mul(t1[:], src[:], cos_sb[:, ssl])
                    t2 = tp.tile([128, 512], BF16, tag="t2", bufs=3, name="rope_t2")
                    nc.vector.tensor_mul(t2[:], shp[:], sin_sb[:, ssl])
                    nc.vector.tensor_add(dst[:, ssl], t1[:], t2[:])

                for sc in range(NSC):
                    ssl = slice(sc * 512, (sc + 1) * 512)
                    for kv in range(KVH):
                        ps = pA.tile([128, 512], F32, tag="qk", name=f"kps{sc}_{kv}")
                        for d in range(DT):
                            nc.tensor.matmul(ps[:], wk_sb[d][:, kv * 128:(kv + 1) * 128],
                                             xts[sc][d][:], start=(d == 0), stop=(d == DT - 1))
                        ktu_t = tp.tile([128, 512], BF16, tag="ktu", bufs=2, name=f"ktu{sc}_{kv}")
                        nc.scalar.copy(ktu_t[:], ps[:])
                        rope_chunk(ktu_t, ktr[kv], ssl, prm)
                    for sv in range(4):
                        st = sc * 4 + sv
                        ps = pAv.tile([128, KW], F32, tag="v", name=f"vps{st}")
                        for d in range(DT):
                            nc.tensor.matmul(ps[:], xts[sc][d][:, sv * 128:(sv + 1) * 128],
                                             wv_sb[d][:], start=(d == 0), stop=(d == DT - 1))
                        nc.scalar.copy(v_sb[st][:], ps[:])
                # Q chunk 3 (needed by attention qc=3, which is processed first)
                ssl3 = slice(3 * 512, 4 * 512)
                for h in range(QH):
                    ps = pA.tile([128, 512], F32, tag="qk", name=f"qps3_{h}")
                    for d in range(DT):
                        nc.tensor.matmul(ps[:], wq_sb[d][:, h * 128:(h + 1) * 128],
                                         xts[3][d][:], start=(d == 0), stop=(d == DT - 1))
                    qtu_t = tp.tile([128, 512], BF16, tag="qtu", bufs=2, name=f"qtu3_{h}")
                    nc.scalar.copy(qtu_t[:], ps[:])
                    rope_chunk(qtu_t, qtr[h], ssl3, prm)

            # ============ Phase B: attention [3,2,1,0] + woven fillers ============
            with (
                tc.tile_pool(name="scp", bufs=2, space="PSUM") as scp,
                tc.tile_pool(name="attnp", bufs=1, space="PSUM") as attnp,
                tc.tile_pool(name="psg", bufs=2, space="PSUM") as psg,
                tc.tile_pool(name="qfp", bufs=1, space="PSUM") as qfp,
            ):
                F = Fillers()

                def gen_q_chunk(sc):
                    """Q projection + rope for s-chunk sc, as filler steps."""
                    ssl = slice(sc * 512, (sc + 1) * 512)
                    for h in range(QH):
                        ps = qfp.tile([128, 512], F32, tag="qf", name=f"qps{sc}_{h}")
                        for d in range(DT):
                            nc.tensor.matmul(ps[:], wq_sb[d][:, h * 128:(h + 1) * 128],
                                             xts[sc][d][:], start=(d == 0), stop=(d == DT - 1))
                            yield
                        qtu_t = tp.tile([128, 512], BF16, tag="qtu", bufs=2,
                                        name=f"qtu{sc}_{h}")
                        nc.vector.tensor_copy(qtu_t[:], ps[:])
                        shp = psg.tile([128, 512], F32, tag="ps", name=f"shp{sc}_{h}")
                        nc.tensor.matmul(shp[:], pswp_sb[:], qtu_t[:], start=True, stop=True)
                        yield
                        t1 = tp.tile([128, 512], BF16, tag="t1", bufs=3, name="rope_t1")
                        nc.vector.tensor_mul(t1[:], qtu_t[:], cos_sb[:, ssl])
                        t2 = tp.tile([128, 512], BF16, tag="t2", bufs=3, name="rope_t2")
                        nc.vector.tensor_mul(t2[:], shp[:], sin_sb[:, ssl])
                        nc.vector.tensor_add(qtr[h][:, ssl], t1[:], t2[:])

                def gen_wo_chunk(qc):
                    """Output projection for q-chunk qc, as filler steps."""
                    qsl = slice(qc * 512, (qc + 1) * 512)
                    for et in range(NET):
                        ps = psg.tile([128, 512], F32, tag="ps", name=f"wops{qc}_{et}")
                        for h in range(QH):
                            nc.tensor.matmul(ps[:], wo_sb[h][:, et * 128:(et + 1) * 128],
                                             attnT[h][:, qsl], start=(h == 0), stop=(h == QH - 1))
                            yield
                        stage = tp.tile([128, 512], BF16, tag="stage", bufs=3,
                                        name=f"stage{qc}_{et}")
                        # alternate copy engine: ACT is exp-saturated mid-kernel
                        if et % 2 == 0:
                            nc.vector.tensor_copy(stage[:], ps[:])
                        else:
                            nc.scalar.copy(stage[:], ps[:])
                        nc.sync.dma_start(outT[et * 128:(et + 1) * 128, qsl], stage[:])

                def attn_chunk(qc):
                    qsl = slice(qc * 512, (qc + 1) * 512)
                    nkt = 4 * qc + 4
                    for h in range(QH):
                        kv = h // 2
                        attn_ps = attnp.tile([128, 512], F32, tag="attn",
                                             name=f"attn{qc}_{h}")
                        dac = tp.tile([128, 512], BF16, tag="dac", bufs=2,
                                      name=f"dac{qc}_{h}")
                        for kt2 in range(0, nkt, 2):
                            sl = scp.tile([128, 1024], F32, tag="sc", name=f"sc{qc}_{h}_{kt2}")
                            offs, spans = [], []
                            for j in range(2):
                                kt = kt2 + j
                                off = max(0, 128 * kt - 512 * qc)
                                span = 512 - off
                                offs.append(off)
                                spans.append(span)
                                nc.tensor.matmul(
                                    sl[:, 512 * j:512 * j + span],
                                    ktr[kv][:, kt * 128:(kt + 1) * 128],
                                    qtr[h][:, qc * 512 + off:(qc + 1) * 512],
                                    start=True, stop=True)
                            F.feed(1)
                            pt = tp.tile([128, 1024], BF16, tag="pt", bufs=4,
                                         name=f"pt{qc}_{h}_{kt2}")
                            if offs[0] == 0:
                                # both spans contiguous from col 0 (bank A full)
                                w = 512 + spans[1]
                                nc.scalar.activation(pt[:, :w], sl[:, :w], Exp, scale=SCALE)
                            else:
                                nc.scalar.activation(pt[:, :spans[0]], sl[:, :spans[0]],
                                                     Exp, scale=SCALE)
                                nc.scalar.activation(pt[:, 512:512 + spans[1]],
                                                     sl[:, 512:512 + spans[1]],
                                                     Exp, scale=SCALE)
                            for j in range(2):
                                kt = kt2 + j
                                if kt >= 4 * qc:  # diagonal block: first 128 of span
                                    nc.vector.tensor_mul(pt[:, 512 * j:512 * j + 128],
                                                         pt[:, 512 * j:512 * j + 128],
                                                         tri_sb[:])
                            for j in range(2):
                                kt = kt2 + j
                                off, span = offs[j], spans[j]
                                nc.tensor.matmul(
                                    attn_ps[:, off:], v_sb[kt][:, kv * 128:(kv + 1) * 128],
                                    pt[:, 512 * j:512 * j + span],
                                    start=(kt == 0), stop=(kt == nkt - 1))
                                if kt == 0:
                                    nc.vector.tensor_copy(dac[:], pt[:, 0:512])
                                else:
                                    nc.vector.tensor_add(dac[:, off:], dac[:, off:],
                                                         pt[:, 512 * j:512 * j + span])
                            F.feed(1)
                        # softmax denominator -> reciprocal -> partition broadcast
                        F.feed(2)
                        den_t = scp.tile([128, 1024], F32, tag="sc", name=f"den{qc}_{h}")
                        nc.tensor.matmul(den_t[0:1, 0:512], ones_sb[:], dac[:],
                                         start=True, stop=True)
                        F.feed(2)
                        recip = tp.tile([1, 512], F32, tag="recip", bufs=2,
                                        name=f"recip{qc}_{h}")
                        nc.vector.reciprocal(recip[:], den_t[0:1, 0:512])
                        recipb = tp.tile([1, 512], BF16, tag="recipb", bufs=2,
                                         name=f"recipb{qc}_{h}")
                        nc.vector.tensor_copy(recipb[:], recip[:])
                        F.feed(1)
                        rb = psg.tile([128, 512], F32, tag="ps", name=f"rb{qc}_{h}")
                        nc.tensor.matmul(rb[:], onesr_sb[:], recipb[:], start=True, stop=True)
                        F.feed(1)
                        # tensor_tensor may read at most one PSUM operand
                        rb_sb = tp.tile([128, 512], F32, tag="rbsb", bufs=2,
                                        name=f"rbsb{qc}_{h}")
                        nc.vector.tensor_copy(rb_sb[:], rb[:])
                        F.feed(1)
                        nc.vector.tensor_mul(attnT[h][:, qsl], attn_ps[:], rb_sb[:])

                qgen = {}
                qgen[2] = F.add(gen_q_chunk(2))
                attn_chunk(3)
                qgen[1] = F.add(gen_q_chunk(1))
                F.add(gen_wo_chunk(3))
                F.drain(qgen[2])  # must be done before qc=2 scores read qtr
                attn_chunk(2)
                qgen[0] = F.add(gen_q_chunk(0))
                F.add(gen_wo_chunk(2))
                F.drain(qgen[1])
                attn_chunk(1)
                F.add(gen_wo_chunk(1))
                F.drain(qgen[0])
                attn_chunk(0)
                F.add(gen_wo_chunk(0))
                F.drain_all()
    return nc


def get_nc():
    if "nc" not in _BUILT:
        nc = bass.Bass("TRN2", debug=False, enable_asserts=False,
                       num_devices=N_CORES)
        _BUILT["nc"] = _build(nc)
    return _BUILT["nc"]


def prepare_in_maps(x, pos_cos, pos_sin, wq, wk, wv, wo):
    bf = ml_dtypes.bfloat16
    x = np.asarray(x, np.float32)
    pos_cos = np.asarray(pos_cos, np.float32)
    pos_sin = np.asarray(pos_sin, np.float32)
    wq = np.asarray(wq, np.float32)
    wk = np.asarray(wk, np.float32)
    wv = np.asarray(wv, np.float32)
    wo = np.asarray(wo, np.float32)

    pair = np.repeat(np.arange(HALF), 2)          # d -> d//2
    C = pos_cos.T[pair]                           # [128, S]
    Sm = pos_sin.T[pair].copy()                   # [128, S]
    Sm[0::2] *= -1.0                              # even d: -sin, odd d: +sin
    pswap = np.zeros((128, 128), np.float32)
    pswap[np.arange(128), np.arange(128) ^ 1] = 1.0
    tri = np.triu(np.ones((128, 128), np.float32))  # keep j >= i (q >= k)
    ones = np.ones((128, 1), np.float32)
    onesr = np.ones((1, 128), np.float32)

    common = {
        "cosb": C.astype(bf), "sinb": Sm.astype(np.float32),
        "pswp": pswap.astype(bf), "tri": tri.astype(bf),
        "ones": ones.astype(bf), "onesr": onesr.astype(bf),
    }
    in_maps = []
    for c in range(N_CORES):
        b, g = divmod(c, 4)
        in_maps.append(dict(
            xt=np.ascontiguousarray(x[b].T).astype(bf),
            wq=wq[:, QW * g:QW * (g + 1)].astype(bf),
            wk=wk[:, KW * g:KW * (g + 1)].astype(bf),
            wv=wv[:, KW * g:KW * (g + 1)].astype(bf),
            wo=wo[QW * g:QW * (g + 1), :].astype(bf),
            **common,
        ))
    return in_maps


def gather(results):
    out = np.zeros((B, S, DIM), np.float32)
    for c in range(N_CORES):
        b = c // 4
        out[b] += results[c]["outT"].astype(np.float32).T
    return out


def run(inputs, trace=False, tmpdir=None):
    nc = get_nc()
    in_maps = prepare_in_maps(**inputs)
    res = run_bass_kernel_spmd(nc, in_maps, list(range(N_CORES)),
                               trace=trace, tmpdir=tmpdir)
    return gather(res.results), res


def kernel(x, pos_cos, pos_sin, wq, wk, wv, wo):
    out, _ = run(dict(x=x, pos_cos=pos_cos, pos_sin=pos_sin,
                      wq=wq, wk=wk, wv=wv, wo=wo))
    return out
